# revision 1
# baseline (speedup 1.0000x reference)
"""GAT encoder (3-layer, 4-head, BN+ELU, mean-pool) on 8 Trainium2 NeuronCores.

Self-contained: host-side planning (edge->slot assignment) + Bass/Tile kernel +
SPMD execution via run_bass_kernel_spmd.

Design:
  - dst-shard nodes across 8 cores (5000/core, padded to 5120 = 40 blocks of 128).
  - Within a core, nodes are sorted by in-degree (desc) into (block, lane): the
    per-lane max over a block ~= the block's mean degree, so lane-aligned slot
    packing is dense.
  - Edge slot (tile, lane): lane = dst's lane; a tile is 128 slots; per block,
    tiles split into A-kind (table rows [0,32768)) and B-kind (rows [8192,40960)
    = offset view) so int16 dma_gather indices cover all 40960 rows; per-lane
    A/B assignment is balanced so pads are rare (pad slots gather row 0 and are
    killed by a -30000 bias before exp).
  - Per layer: h|al_src|al_dst = x @ [W*bnscale | W@Asrc | W@Adst] per block
    (PE transpose + matmul), rows -> local slab -> AllGather -> bf16 table
    [40960, 256]; edge phase gathers 512B rows by src, computes
    p = exp(leakyrelu(al_src + al_dst) + padbias), weights h by p, and
    accumulates [h*p | p] into PSUM via identity-lhsT matmuls (segment sum);
    flush normalizes by s (the softmax max-shift cancels; s+1e-16 also guards
    padded window rows), adds BN shift, applies ELU.
  - Mean-pool partials [64, 128] per core via one-hot matmuls; host sums across
    cores and divides by counts.
"""
import sys

sys.path.insert(0, "/opt/trn_rl_repo")

import numpy as np
from concourse import bass, mybir, tile, bacc
from concourse.bass_utils import run_bass_kernel_spmd

f32 = mybir.dt.float32
bf16 = mybir.dt.bfloat16
i16 = mybir.dt.int16

P = 128
NEG_SLOPE = 0.2
BN_EPS = 1e-5
PAD_BIAS = -30000.0

CFG_FULL = dict(N=40000, E=640000, D=128, H=4, L=3, G=64, CORES=8)


def _cfg_derived(cfg):
    cores = cfg["CORES"]
    npc = cfg["N"] // cores              # real nodes per core
    npad = -(-npc // P) * P              # padded nodes per core
    blocks = npad // P
    rows = npad * cores                  # global table rows
    lo_max = 32768                       # A-kind covers [0, lo_max)
    b_off = rows - 32768 if rows > 32768 else 0   # B-kind covers [b_off, rows)
    return npc, npad, blocks, rows, lo_max, b_off


def plan(cfg, edge_index, batch):
    """Host planning. Returns (sched, percore) where sched is core-uniform."""
    N, E, G = cfg["N"], cfg["E"], cfg["G"]
    cores = cfg["CORES"]
    npc, npad, blocks, rows, lo_max, b_off = _cfg_derived(cfg)

    src = np.asarray(edge_index[0], dtype=np.int64)
    dst = np.asarray(edge_index[1], dtype=np.int64)
    loops = np.arange(N, dtype=np.int64)
    src = np.concatenate([src, loops])
    dst = np.concatenate([dst, loops])
    batch = np.asarray(batch, dtype=np.int64)

    deg = np.bincount(dst, minlength=N)

    # node -> (core, block, lane); within a core sort by degree desc
    core_of = dst // npc  # for edges
    node_core = np.arange(N) // npc
    pos = np.empty(N, dtype=np.int64)       # position within core (block*128+lane)
    for c in range(cores):
        nodes = np.arange(c * npc, (c + 1) * npc)
        order = nodes[np.argsort(-deg[nodes], kind="stable")]
        pos[order] = np.arange(npc)
    remap = node_core * npad + pos          # node -> global table row

    src_r = remap[src]                      # gather row of each edge
    dst_c = core_of                         # owning core of each edge
    dst_b = pos[dst] // P                   # block within core
    dst_w = pos[dst] % P                    # lane

    # group edges by (core, block, lane)
    key = (dst_c * blocks + dst_b) * P + dst_w
    order = np.argsort(key, kind="stable")
    src_r_s = src_r[order]
    key_s = key[order]
    grp_start = np.searchsorted(key_s, np.arange(cores * blocks * P))
    grp_end = np.searchsorted(key_s, np.arange(cores * blocks * P) + 1)

    # per (core, block, lane): how many edges must be A (src_row < lo_max can be
    # A; src_row >= b_off can be B). mustA: src_row < b_off; mustB: >= lo_max.
    is_mustA = src_r_s < b_off
    is_mustB = src_r_s >= lo_max
    mustA = np.zeros(cores * blocks * P, np.int64)
    mustB = np.zeros(cores * blocks * P, np.int64)
    np.add.at(mustA, key_s, is_mustA)
    np.add.at(mustB, key_s, is_mustB)
    cnt = grp_end - grp_start

    mustA = mustA.reshape(cores, blocks, P)
    mustB = mustB.reshape(cores, blocks, P)
    cntr = cnt.reshape(cores, blocks, P)

    # choose per-block (shared across cores) k_A, k_B minimizing pads
    kA = np.zeros(blocks, np.int64)
    kB = np.zeros(blocks, np.int64)
    for b in range(blocks):
        mA, mB, cc = mustA[:, b], mustB[:, b], cntr[:, b]
        best = None
        lo = int(mA.max())
        hi = int(np.maximum(cc - mB, mA).max())
        for ka in range(lo, hi + 1):
            nA = np.clip(ka, mA, np.maximum(cc - mB, mA))
            nA = np.minimum(nA, ka)  # lane can't use more A slots than exist
            nA = np.maximum(nA, np.minimum(mA, ka))
            # feasibility: every lane must fit its edges: nB = cc - nA <= kb
            nB = cc - nA
            kb = int(nB.max())
            pads = (ka - nA).sum() + (kb - nB).sum()
            if best is None or pads < best[0]:
                best = (pads, ka, kb)
        _, ka, kb = best
        kA[b], kB[b] = ka, kb

    # global tile layout: superchunks of SC blocks; within: A tiles of the
    # blocks (in block order), then B tiles of the blocks.
    SC = 2
    tiles = []          # list of (block, kind)
    gathers = []        # list of (kind, tile_lo, tile_hi)  [tile indices into `tiles`]
    t = 0
    for s0 in range(0, blocks, SC):
        bl = list(range(s0, min(s0 + SC, blocks)))
        a0 = t
        for b in bl:
            tiles += [(b, 0)] * int(kA[b])
        t = len(tiles)
        if t > a0:
            gathers.append((0, a0, t))
        b0 = t
        for b in bl:
            tiles += [(b, 1)] * int(kB[b])
        t = len(tiles)
        if t > b0:
            gathers.append((1, b0, t))
    T_TOT = len(tiles)
    tile_block = np.array([b for b, _ in tiles], np.int64)
    # first/last tile per block
    first = {}
    last = {}
    for i, (b, _) in enumerate(tiles):
        if b not in first:
            first[b] = i
        last[b] = i

    # z-chain segments: runs of equal block in tile order
    segs = []  # (tile_lo, tile_hi, block)
    i = 0
    while i < T_TOT:
        j = i
        while j < T_TOT and tile_block[j] == tile_block[i]:
            j += 1
        segs.append((i, j, int(tile_block[i])))
        i = j

    # per-gather idx column offsets (in int16 columns, each tile -> 8 columns)
    gmeta = []
    colA = colB = 0
    for kind, lo, hi in gathers:
        nt = hi - lo
        if kind == 0:
            gmeta.append((kind, lo, nt, colA))
            colA += nt * 8
        else:
            gmeta.append((kind, lo, nt, colB))
            colB += nt * 8

    sched = dict(
        T_TOT=T_TOT, tiles=tiles, gathers=gmeta, segs=segs,
        first=first, last=last, kA=kA, kB=kB,
        colsA=colA, colsB=colB, blocks=blocks, npad=npad, rows=rows,
        b_off=b_off,
    )

    # ---------- per-core data ----------
    percore = []
    for c in range(cores):
        idxA = np.zeros((16, colA), np.int16)
        idxB = np.zeros((16, colB), np.int16)
        padb = np.full((P, T_TOT), PAD_BIAS, np.float32)
        # slot fill: per block, per lane: A-edges then B-edges of that lane
        # (choose nA per lane as planned)
        mA, mB, cc = mustA[c], mustB[c], cntr[c]
        for kind, lo, nt, col in gmeta:
            flat = np.zeros(nt * P, np.int64)   # default pad -> row 0
            valid = np.zeros(nt * P, bool)
            # local tile index within this gather per global tile
            for ti in range(nt):
                gt = lo + ti
                b = int(tile_block[gt])
                # tile position within its block's kind-run
                # count tiles of same (block, kind) before gt
                flat_ti = ti  # not used
            # fill lane-by-lane using group lists
            # For block b: its A tiles are the kA[b] tiles of kind 0 with block b,
            # in order; j-th A tile holds lane w's j-th A-edge.
            # Precompute per-block tile positions inside this gather:
            btiles = {}
            for ti in range(nt):
                b = int(tile_block[lo + ti])
                btiles.setdefault(b, []).append(ti)
            for b, tl in btiles.items():
                ka = int(kA[b])
                for w in range(P):
                    g0 = grp_start[(c * blocks + b) * P + w]
                    g1 = grp_end[(c * blocks + b) * P + w]
                    edges = src_r_s[g0:g1]
                    na = int(np.clip(ka, mA[b, w], max(cc[b, w] - mB[b, w], mA[b, w])))
                    na = min(na, ka, cc[b, w])
                    # ensure all non-A edges are B-eligible: put mustA first
                    ea = edges[edges < lo_max]
                    eb = edges[edges >= lo_max]
                    # A slots take from ea (must include all ea-only edges if
                    # B-ineligible). edges < b_off are A-only.
                    a_only = edges[edges < b_off]
                    both = edges[(edges >= b_off) & (edges < lo_max)]
                    b_only = eb
                    take_a = list(a_only) + list(both[: na - len(a_only)])
                    take_b = list(both[max(0, na - len(a_only)):]) + list(b_only)
                    if kind == 0:
                        for j, e in enumerate(take_a):
                            flat[tl[j] * P + w] = e
                            valid[tl[j] * P + w] = True
                    else:
                        for j, e in enumerate(take_b):
                            flat[tl[j] * P + w] = e - b_off
                            valid[tl[j] * P + w] = True
            # wrapped-16 layout
            wr = flat.reshape(nt * 8, 16).T.astype(np.int16)
            if kind == 0:
                idxA[:, col : col + nt * 8] = wr
            else:
                idxB[:, col : col + nt * 8] = wr
            # padbias (by global tile / lane)
            v = valid.reshape(nt, P)
            for ti in range(nt):
                gt = lo + ti
                padb[v[ti], gt] = 0.0

        percore.append(dict(
            idxA=np.tile(idxA, (8, 1)),
            idxB=np.tile(idxB, (8, 1)),
            padb=padb,
        ))

    # batch per (lane, block) and x permutation
    inv = np.empty(cores * npad, np.int64)  # table row -> orig node (or -1)
    inv[:] = -1
    inv[remap] = np.arange(N)
    for c in range(cores):
        rowsl = inv[c * npad : (c + 1) * npad]
        bb = np.full(npad, float(G), np.float32)
        ok = rowsl >= 0
        bb[ok] = batch[rowsl[ok]].astype(np.float32)
        percore[c]["batchb"] = bb.reshape(blocks, P).T.copy()  # [P, blocks]
        percore[c]["perm"] = rowsl                              # for x shard
    cnt_g = np.bincount(batch, minlength=G).astype(np.float32)
    sched["cnt_g"] = cnt_g
    return sched, percore


def build(cfg, sched, reps=None, parts="all"):
    N, D, H, L, G = cfg["N"], cfg["D"], cfg["H"], cfg["L"], cfg["G"]
    cores = cfg["CORES"]
    npc, npad, blocks, rows, lo_max, b_off = _cfg_derived(cfg)
    T_TOT = sched["T_TOT"]
    RW = 2 * D  # table row elems (bf16): [h(D) | al_src(H) | pad]
    assert RW % 128 == 0

    nc = bacc.Bacc("TRN2", target_bir_lowering=False, debug=False, num_devices=cores,
                   num_swdge_queues=2)

    xs = nc.dram_tensor("xs", [npad, D], f32, kind="ExternalInput")
    wcat = nc.dram_tensor("wcat", [L, D, D + 2 * H], f32, kind="ExternalInput")
    bnsh = nc.dram_tensor("bnsh", [L, P, D], f32, kind="ExternalInput")
    idxA = nc.dram_tensor("idxA", [P, max(sched["colsA"], 8)], i16, kind="ExternalInput")
    idxB = nc.dram_tensor("idxB", [P, max(sched["colsB"], 8)], i16, kind="ExternalInput")
    padb = nc.dram_tensor("padb", [P, T_TOT], f32, kind="ExternalInput")
    batb = nc.dram_tensor("batchb", [P, blocks], f32, kind="ExternalInput")
    iog = nc.dram_tensor("iog", [P, G], f32, kind="ExternalInput")
    identb = nc.dram_tensor("identb", [P, P], bf16, kind="ExternalInput")
    identf = nc.dram_tensor("identf", [P, P], f32, kind="ExternalInput")
    outp = nc.dram_tensor("out", [G, D], f32, kind="ExternalOutput")

    slab = nc.dram_tensor("slab", [npad, RW], bf16)
    table = nc.dram_tensor("table", [rows, RW], bf16, addr_space="Shared")

    NAL = D + 2 * H  # 144 for D=128,H=4

    with tile.TileContext(nc) as tc:
        # ---- persistent SBUF ----
        x_sb = nc.alloc_sbuf_tensor("x_sb", [P, blocks * D], f32)
        ad_sb = nc.alloc_sbuf_tensor("ad_sb", [P, blocks * H], f32)  # al_dst per (lane, block)
        wc_sb = nc.alloc_sbuf_tensor("wc_sb", [P, L * NAL], f32)
        sh_sb = nc.alloc_sbuf_tensor("sh_sb", [P, L * D], f32)
        bat_sb = nc.alloc_sbuf_tensor("bat_sb", [P, blocks], f32)
        iog_sb = nc.alloc_sbuf_tensor("iog_sb", [P, G], f32)
        idb_sb = nc.alloc_sbuf_tensor("idb_sb", [P, P], bf16)
        idf_sb = nc.alloc_sbuf_tensor("idf_sb", [P, P], f32)
        pb_sb = nc.alloc_sbuf_tensor("pb_sb", [P, T_TOT], f32)

        nc.sync.dma_start(out=x_sb[:].rearrange("p (b f) -> p b f", f=D),
                          in_=xs[:, :].rearrange("(b p) f -> p b f", p=P))
        for l in range(L):
            nc.sync.dma_start(out=wc_sb[:, l * NAL : (l + 1) * NAL], in_=wcat[l])
            nc.sync.dma_start(out=sh_sb[:, l * D : (l + 1) * D], in_=bnsh[l])
        nc.sync.dma_start(out=bat_sb[:], in_=batb[:, :])
        nc.sync.dma_start(out=iog_sb[:], in_=iog[:, :])
        nc.sync.dma_start(out=idb_sb[:], in_=identb[:, :])
        nc.sync.dma_start(out=idf_sb[:], in_=identf[:, :])
        nc.sync.dma_start(out=pb_sb[:], in_=padb[:, :])

        from contextlib import nullcontext
        with (
            tc.tile_pool(name="sb", bufs=2) as sb,
            tc.tile_pool(name="sbg", bufs=2) as sbg,
            tc.tile_pool(name="ps", bufs=2, space="PSUM") as ps,
            tc.tile_pool(name="psacc", bufs=3, space="PSUM") as psacc,
            tc.tile_pool(name="pspool", bufs=1, space="PSUM") as pspool,
        ):
            def phase1_ag(l):
                for b in range(blocks):
                    xT_p = ps.tile([P, P], f32, tag="xT_p")
                    nc.tensor.transpose(out=xT_p[:], in_=x_sb[:, b * D : (b + 1) * D],
                                        identity=idf_sb[:])
                    xT_s = sb.tile([P, P], f32, tag="xT_s")
                    nc.vector.tensor_copy(out=xT_s[:], in_=xT_p[:])
                    hrow = ps.tile([P, NAL], f32, tag="hrow")
                    nc.tensor.matmul(hrow[:], lhsT=xT_s[:],
                                     rhs=wc_sb[:, l * NAL : (l + 1) * NAL],
                                     start=True, stop=True)
                    strow = sb.tile([P, RW], bf16, tag="strow")
                    nc.scalar.memzero(strow[:, D + H :])
                    nc.vector.tensor_copy(out=strow[:, 0 : D + H], in_=hrow[:, 0 : D + H])
                    nc.vector.tensor_copy(out=ad_sb[:, b * H : (b + 1) * H],
                                          in_=hrow[:, D + H : D + 2 * H])
                    nc.sync.dma_start(out=slab[b * P : (b + 1) * P, :], in_=strow[:])
                nc.gpsimd.collective_compute(
                    "AllGather", mybir.AluOpType.bypass,
                    replica_groups=[list(range(cores))],
                    ins=[slab[:, :].opt()], outs=[table[:, :].opt()],
                )
            if reps is not None:
                for l in range(L):
                    phase1_ag(l)
                loop_cm = tc.For_i(0, reps, 1)
            else:
                loop_cm = nullcontext()
            with loop_cm:
                for l in range(L):
                    # phase 1 (+ AG in normal mode; timing mode: phase1 only,
                    # AGs already done outside the loop)
                    for b in (range(blocks) if parts in ("all", "p1") else []):
                        xT_p = ps.tile([P, P], f32, tag="xT_p")
                        nc.tensor.transpose(out=xT_p[:], in_=x_sb[:, b * D : (b + 1) * D],
                                            identity=idf_sb[:])
                        xT_s = sb.tile([P, P], f32, tag="xT_s")
                        nc.vector.tensor_copy(out=xT_s[:], in_=xT_p[:])
                        hrow = ps.tile([P, NAL], f32, tag="hrow")
                        nc.tensor.matmul(hrow[:], lhsT=xT_s[:],
                                         rhs=wc_sb[:, l * NAL : (l + 1) * NAL],
                                         start=True, stop=True)
                        strow = sb.tile([P, RW], bf16, tag="strow")
                        nc.scalar.memzero(strow[:, D + H :])
                        nc.vector.tensor_copy(out=strow[:, 0 : D + H], in_=hrow[:, 0 : D + H])
                        nc.vector.tensor_copy(out=ad_sb[:, b * H : (b + 1) * H],
                                              in_=hrow[:, D + H : D + 2 * H])
                        nc.sync.dma_start(out=slab[b * P : (b + 1) * P, :], in_=strow[:])
                    if reps is None:
                        nc.gpsimd.collective_compute(
                            "AllGather", mybir.AluOpType.bypass,
                            replica_groups=[list(range(cores))],
                            ins=[slab[:, :].opt()], outs=[table[:, :].opt()],
                        )

                    if parts == "p1":
                        continue
                    if parts == "gather":
                        gdummy = sb.tile([P, 4], f32, tag="gdummy")
                        nc.vector.tensor_copy(out=gdummy[:], in_=x_sb[:, 0:4])
                    # ---------- edge phase ----------
                    # one G buffer per gather (per kind) so A/B run concurrently
                    # on separate SWDGE queues.
                    acc = {}          # block -> psum tile
                    gathers = sched["gathers"]
                    segs = sched["segs"]
                    for kind, lo, nt, col in gathers:
                        sc_hi = lo + nt
                        it = sb.tile([P, nt * 8], i16, tag=f"it{kind}")
                        src_idx = idxA if kind == 0 else idxB
                        nc.sync.dma_start(out=it[:], in_=src_idx[:, col : col + nt * 8])
                        in_ap = table[:, :] if kind == 0 else table[b_off:, :]
                        Gk = sbg.tile([P, nt * RW], bf16, tag=f"G{kind}")
                        nc.gpsimd.dma_gather(
                            out_ap=Gk[:].rearrange("p (j w) -> p j w", w=RW),
                            in_ap=in_ap,
                            idxs_ap=it[:],
                            num_idxs=nt * P,
                            num_idxs_reg=nt * P,
                            elem_size=RW,
                            single_packet=False,
                            queue_num=kind,
                        )
                        if parts == "gather":
                            nc.vector.tensor_tensor(out=gdummy[:], in0=gdummy[:],
                                                    in1=Gk[:, 0:4],
                                                    op=mybir.AluOpType.add)
                            continue
                        Z = sb.tile([P, nt * H], f32, tag=f"Z{kind}")
                        Gv = Gk[:].rearrange("p (t w) -> p t w", w=RW)
                        Zv = Z[:].rearrange("p (t h) -> p t h", h=H)
                        for (s_lo, s_hi, blk) in segs:
                            if s_hi <= lo or s_lo >= sc_hi:
                                continue
                            a, bnd = max(s_lo, lo) - lo, min(s_hi, sc_hi) - lo
                            nc.vector.tensor_tensor(
                                out=Zv[:, a:bnd, :],
                                in0=Gv[:, a:bnd, D : D + H],
                                in1=ad_sb[:, blk * H : (blk + 1) * H][:, None, :]
                                    .to_broadcast([P, bnd - a, H]),
                                op=mybir.AluOpType.add,
                            )
                        nc.vector.tensor_tensor(
                            out=Zv[:, :, :], in0=Zv[:, :, :],
                            in1=pb_sb[:, lo:sc_hi][:, :, None].to_broadcast([P, nt, H]),
                            op=mybir.AluOpType.add,
                        )
                        nc.vector.scalar_tensor_tensor(
                            out=Z[:], in0=Z[:], scalar=NEG_SLOPE, in1=Z[:],
                            op0=mybir.AluOpType.mult, op1=mybir.AluOpType.max,
                        )
                        nc.scalar.activation(out=Gv[:, :, D : D + H], in_=Zv[:, :, :],
                                             func=mybir.ActivationFunctionType.Exp)
                        nc.vector.tensor_tensor(
                            out=Gv[:, :, 0:D].rearrange("p t (h c) -> p t h c", h=H),
                            in0=Gv[:, :, 0:D].rearrange("p t (h c) -> p t h c", h=H),
                            in1=Gv[:, :, D : D + H][:, :, :, None]
                                .to_broadcast([P, nt, H, D // H]),
                            op=mybir.AluOpType.mult,
                        )
                        # matmuls + flushes for this gather's tiles
                        for t in range(lo, sc_hi):
                            blk = int(sched["tiles"][t][0])
                            if sched["first"][blk] == t:
                                acc_t = psacc.tile([P, D + H], f32, tag="acc")
                                acc[blk] = acc_t
                            nc.tensor.matmul(
                                acc[blk][:],
                                lhsT=idb_sb[:],
                                rhs=Gk[:, (t - lo) * RW : (t - lo) * RW + D + H],
                                start=(sched["first"][blk] == t),
                                stop=(sched["last"][blk] == t),
                            )
                            if sched["last"][blk] == t:
                                a = acc.pop(blk)
                                # flush: normalize + shift + ELU -> x_sb
                                s4 = sb.tile([P, H], f32, tag="s4")
                                nc.vector.tensor_scalar(
                                    out=s4[:], in0=a[:, D : D + H],
                                    scalar1=1e-16, scalar2=None,
                                    op0=mybir.AluOpType.add,
                                )
                                r4 = sb.tile([P, H], f32, tag="r4")
                                nc.vector.reciprocal(out=r4[:], in_=s4[:])
                                xb = x_sb[:, blk * D : (blk + 1) * D]
                                t0 = sb.tile([P, D], f32, tag="t0")
                                nc.vector.tensor_tensor(
                                    out=t0[:].rearrange("p (h c) -> p h c", h=H),
                                    in0=a[:, 0:D].rearrange("p (h c) -> p h c", h=H),
                                    in1=r4[:][:, :, None].to_broadcast([P, H, D // H]),
                                    op=mybir.AluOpType.mult,
                                )
                                # t1 = t0 + shift
                                nc.vector.tensor_tensor(
                                    out=t0[:], in0=t0[:],
                                    in1=sh_sb[:, l * D : (l + 1) * D],
                                    op=mybir.AluOpType.add,
                                )
                                # ELU: x = (max(t1,0)-1) + exp(min(t1,0))
                                mneg = sb.tile([P, D], f32, tag="mneg")
                                nc.vector.tensor_scalar(
                                    out=mneg[:], in0=t0[:], scalar1=0.0, scalar2=None,
                                    op0=mybir.AluOpType.min,
                                )
                                nc.scalar.activation(out=mneg[:], in_=mneg[:],
                                                     func=mybir.ActivationFunctionType.Exp)
                                nc.vector.tensor_scalar(
                                    out=xb, in0=t0[:], scalar1=0.0, scalar2=-1.0,
                                    op0=mybir.AluOpType.max, op1=mybir.AluOpType.add,
                                )
                                nc.vector.tensor_tensor(
                                    out=xb, in0=xb, in1=mneg[:], op=mybir.AluOpType.add,
                                )

            # ---------- mean-pool partials ----------
            pacc = pspool.tile([G, D], f32, tag="pacc")
            for b in range(blocks):
                sp = sb.tile([P, G], f32, tag="sp")
                nc.vector.tensor_tensor(
                    out=sp[:],
                    in0=bat_sb[:, b : b + 1].to_broadcast([P, G]),
                    in1=iog_sb[:],
                    op=mybir.AluOpType.is_equal,
                )
                nc.tensor.matmul(pacc[:], lhsT=sp[:], rhs=x_sb[:, b * D : (b + 1) * D],
                                 start=(b == 0), stop=(b == blocks - 1))
            po = sb.tile([G, D], f32, tag="po")
            nc.vector.tensor_copy(out=po[:], in_=pacc[:])
            nc.sync.dma_start(out=outp[:, :], in_=po[:])

    nc.compile()
    return nc


def _host_params(cfg, Ws, att_src, att_dst, bias, bn_gamma, bn_beta, bn_mean, bn_var):
    L, D, H = cfg["L"], cfg["D"], cfg["H"]
    C = D // H
    wcat = np.zeros((L, D, D + 2 * H), np.float32)
    bnsh = np.zeros((L, P, D), np.float32)
    for l in range(L):
        sc = bn_gamma[l] / np.sqrt(bn_var[l] + BN_EPS)
        sh = (bias[l] - bn_mean[l]) * sc + bn_beta[l]
        As = np.zeros((D, H), np.float32)
        Ad = np.zeros((D, H), np.float32)
        for h in range(H):
            As[h * C : (h + 1) * C, h] = att_src[l, h]
            Ad[h * C : (h + 1) * C, h] = att_dst[l, h]
        wcat[l, :, :D] = Ws[l] * sc[None, :]
        wcat[l, :, D : D + H] = Ws[l] @ As
        wcat[l, :, D + H : D + 2 * H] = Ws[l] @ Ad
        bnsh[l, :, :] = np.tile(sh[None, :], (P, 1))
    return wcat, bnsh


_BUILD_CACHE = {}


def run_gat(cfg, inputs, nc=None, sched=None, percore=None):
    """Full pipeline on a given cfg. Returns (out, nc, sched, percore)."""
    N, D, G = cfg["N"], cfg["D"], cfg["G"]
    cores = cfg["CORES"]
    npc, npad, blocks, rows, lo_max, b_off = _cfg_derived(cfg)

    x = np.asarray(inputs["x"], np.float32)
    if sched is None:
        sched, percore = plan(cfg, np.asarray(inputs["edge_index"]),
                              np.asarray(inputs["batch"]))
    if nc is None:
        nc = build(cfg, sched)

    wcat, bnsh = _host_params(
        cfg, np.asarray(inputs["Ws"], np.float32),
        np.asarray(inputs["att_src"], np.float32),
        np.asarray(inputs["att_dst"], np.float32),
        np.asarray(inputs["bias"], np.float32),
        np.asarray(inputs["bn_gamma"], np.float32),
        np.asarray(inputs["bn_beta"], np.float32),
        np.asarray(inputs["bn_mean"], np.float32),
        np.asarray(inputs["bn_var"], np.float32),
    )
    iog = np.tile(np.arange(G, dtype=np.float32)[None, :], (P, 1))
    identf = np.eye(P, dtype=np.float32)
    import jax.numpy as jnp
    identb = np.asarray(jnp.asarray(identf, jnp.bfloat16))

    in_maps = []
    for c in range(cores):
        pc = percore[c]
        xs = np.zeros((npad, D), np.float32)
        ok = pc["perm"] >= 0
        xs[ok] = x[pc["perm"][ok]]
        in_maps.append(dict(
            xs=xs, wcat=wcat, bnsh=bnsh,
            idxA=pc["idxA"] if pc["idxA"].shape[1] else np.zeros((P, 8), np.int16),
            idxB=pc["idxB"] if pc["idxB"].shape[1] else np.zeros((P, 8), np.int16),
            padb=pc["padb"], batchb=pc["batchb"].astype(np.float32),
            iog=iog, identb=identb, identf=identf,
        ))
    res = run_bass_kernel_spmd(nc, in_maps, list(range(cores)))
    parts = np.stack([res.results[c]["out"] for c in range(cores)])
    out = parts.sum(axis=0) / np.maximum(sched["cnt_g"], 1.0)[:, None]
    return out.astype(np.float32), nc, sched, percore


def kernel(**inputs) -> np.ndarray:
    out, *_ = run_gat(CFG_FULL, inputs)
    return out



# revision 7
# speedup vs baseline: 467.7051x; 467.7051x over previous
"""GAT encoder (3-layer, 4-head, BN+ELU, mean-pool) on 8 Trainium2 NeuronCores.

Self-contained: host-side planning (edge->slot assignment) + Bass/Tile kernel +
SPMD execution via run_bass_kernel_spmd.

Design:
  - dst-shard nodes across 8 cores (5000/core, padded to 5120 = 40 blocks of 128).
  - Within a core, nodes are sorted by in-degree (desc) into (block, lane): the
    per-lane max over a block ~= the block's mean degree, so lane-aligned slot
    packing is dense.
  - Edge slot (tile, lane): lane = dst's lane; a tile is 128 slots; per block,
    tiles split into A-kind (table rows [0,32768)) and B-kind (rows [8192,40960)
    = offset view) so int16 dma_gather indices cover all 40960 rows; per-lane
    A/B assignment is balanced so pads are rare (pad slots gather row 0 and are
    killed by a -30000 bias before exp).
  - Per layer: h|al_src|al_dst = x @ [W*bnscale | W@Asrc | W@Adst] per block
    (PE transpose + matmul), rows -> local slab -> AllGather -> bf16 table
    [40960, 256]; edge phase gathers 512B rows by src, computes
    p = exp(leakyrelu(al_src + al_dst) + padbias), weights h by p, and
    accumulates [h*p | p] into PSUM via identity-lhsT matmuls (segment sum);
    flush normalizes by s (the softmax max-shift cancels; s+1e-16 also guards
    padded window rows), adds BN shift, applies ELU.
  - Mean-pool partials [64, 128] per core via one-hot matmuls; host sums across
    cores and divides by counts.
"""
import sys

sys.path.insert(0, "/opt/trn_rl_repo")

import numpy as np
from concourse import bass, mybir, tile, bacc
from concourse.bass_utils import run_bass_kernel_spmd

f32 = mybir.dt.float32
bf16 = mybir.dt.bfloat16
i16 = mybir.dt.int16

P = 128
NEG_SLOPE = 0.2
BN_EPS = 1e-5
PAD_BIAS = -30000.0

CFG_FULL = dict(N=40000, E=640000, D=128, H=4, L=3, G=64, CORES=8)


def _cfg_derived(cfg):
    cores = cfg["CORES"]
    npc = cfg["N"] // cores              # real nodes per core
    npad = -(-npc // P) * P              # padded nodes per core
    blocks = npad // P
    rows = npad * cores                  # global table rows
    lo_max = 32768                       # A-kind covers [0, lo_max)
    b_off = rows - 32768 if rows > 32768 else 0   # B-kind covers [b_off, rows)
    return npc, npad, blocks, rows, lo_max, b_off


def plan(cfg, edge_index, batch):
    """Host planning. Returns (sched, percore) where sched is core-uniform."""
    N, E, G = cfg["N"], cfg["E"], cfg["G"]
    cores = cfg["CORES"]
    npc, npad, blocks, rows, lo_max, b_off = _cfg_derived(cfg)

    src = np.asarray(edge_index[0], dtype=np.int64)
    dst = np.asarray(edge_index[1], dtype=np.int64)
    loops = np.arange(N, dtype=np.int64)
    src = np.concatenate([src, loops])
    dst = np.concatenate([dst, loops])
    batch = np.asarray(batch, dtype=np.int64)

    deg = np.bincount(dst, minlength=N)

    # node -> (core, block, lane); within a core sort by degree desc
    core_of = dst // npc  # for edges
    node_core = np.arange(N) // npc
    pos = np.empty(N, dtype=np.int64)       # position within core (block*128+lane)
    for c in range(cores):
        nodes = np.arange(c * npc, (c + 1) * npc)
        order = nodes[np.argsort(-deg[nodes], kind="stable")]
        pos[order] = np.arange(npc)
    remap = node_core * npad + pos          # node -> global table row

    src_r = remap[src]                      # gather row of each edge
    dst_c = core_of                         # owning core of each edge
    dst_b = pos[dst] // P                   # block within core
    dst_w = pos[dst] % P                    # lane

    # group edges by (core, block, lane)
    key = (dst_c * blocks + dst_b) * P + dst_w
    order = np.argsort(key, kind="stable")
    src_r_s = src_r[order]
    key_s = key[order]
    grp_start = np.searchsorted(key_s, np.arange(cores * blocks * P))
    grp_end = np.searchsorted(key_s, np.arange(cores * blocks * P) + 1)

    # per (core, block, lane): how many edges must be A (src_row < lo_max can be
    # A; src_row >= b_off can be B). mustA: src_row < b_off; mustB: >= lo_max.
    is_mustA = src_r_s < b_off
    is_mustB = src_r_s >= lo_max
    mustA = np.zeros(cores * blocks * P, np.int64)
    mustB = np.zeros(cores * blocks * P, np.int64)
    np.add.at(mustA, key_s, is_mustA)
    np.add.at(mustB, key_s, is_mustB)
    cnt = grp_end - grp_start

    mustA = mustA.reshape(cores, blocks, P)
    mustB = mustB.reshape(cores, blocks, P)
    cntr = cnt.reshape(cores, blocks, P)

    # choose per-block (shared across cores) k_A, k_B minimizing pads
    kA = np.zeros(blocks, np.int64)
    kB = np.zeros(blocks, np.int64)
    for b in range(blocks):
        mA, mB, cc = mustA[:, b], mustB[:, b], cntr[:, b]
        best = None
        lo = int(mA.max())
        hi = int(np.maximum(cc - mB, mA).max())
        for ka in range(lo, hi + 1):
            nA = np.clip(ka, mA, np.maximum(cc - mB, mA))
            nA = np.minimum(nA, ka)  # lane can't use more A slots than exist
            nA = np.maximum(nA, np.minimum(mA, ka))
            # feasibility: every lane must fit its edges: nB = cc - nA <= kb
            nB = cc - nA
            kb = int(nB.max())
            pads = (ka - nA).sum() + (kb - nB).sum()
            if best is None or pads < best[0]:
                best = (pads, ka, kb)
        _, ka, kb = best
        kA[b], kB[b] = ka, kb

    # global tile layout: superchunks of SC blocks; within: A tiles of the
    # blocks (in block order), then B tiles of the blocks.
    SC = 2
    tiles = []          # list of (block, kind)
    gathers = []        # list of (kind, tile_lo, tile_hi)  [tile indices into `tiles`]
    t = 0
    for s0 in range(0, blocks, SC):
        bl = list(range(s0, min(s0 + SC, blocks)))
        a0 = t
        for b in bl:
            tiles += [(b, 0)] * int(kA[b])
        t = len(tiles)
        if t > a0:
            gathers.append((0, a0, t))
        b0 = t
        for b in bl:
            tiles += [(b, 1)] * int(kB[b])
        t = len(tiles)
        if t > b0:
            gathers.append((1, b0, t))
    T_TOT = len(tiles)
    tile_block = np.array([b for b, _ in tiles], np.int64)
    # first/last tile per block
    first = {}
    last = {}
    for i, (b, _) in enumerate(tiles):
        if b not in first:
            first[b] = i
        last[b] = i

    # z-chain segments: runs of equal block in tile order
    segs = []  # (tile_lo, tile_hi, block)
    i = 0
    while i < T_TOT:
        j = i
        while j < T_TOT and tile_block[j] == tile_block[i]:
            j += 1
        segs.append((i, j, int(tile_block[i])))
        i = j

    # per-gather idx column offsets (in int16 columns, each tile -> 8 columns)
    gmeta = []
    colA = colB = 0
    for kind, lo, hi in gathers:
        nt = hi - lo
        if kind == 0:
            gmeta.append((kind, lo, nt, colA))
            colA += nt * 8
        else:
            gmeta.append((kind, lo, nt, colB))
            colB += nt * 8

    sched = dict(
        T_TOT=T_TOT, tiles=tiles, gathers=gmeta, segs=segs,
        first=first, last=last, kA=kA, kB=kB,
        colsA=colA, colsB=colB, blocks=blocks, npad=npad, rows=rows,
        b_off=b_off,
    )

    # ---------- per-core data ----------
    percore = []
    for c in range(cores):
        idxA = np.zeros((16, colA), np.int16)
        idxB = np.zeros((16, colB), np.int16)
        padb = np.full((P, T_TOT), PAD_BIAS, np.float32)
        # slot fill: per block, per lane: A-edges then B-edges of that lane
        # (choose nA per lane as planned)
        mA, mB, cc = mustA[c], mustB[c], cntr[c]
        for kind, lo, nt, col in gmeta:
            flat = np.zeros(nt * P, np.int64)   # default pad -> row 0
            valid = np.zeros(nt * P, bool)
            # local tile index within this gather per global tile
            for ti in range(nt):
                gt = lo + ti
                b = int(tile_block[gt])
                # tile position within its block's kind-run
                # count tiles of same (block, kind) before gt
                flat_ti = ti  # not used
            # fill lane-by-lane using group lists
            # For block b: its A tiles are the kA[b] tiles of kind 0 with block b,
            # in order; j-th A tile holds lane w's j-th A-edge.
            # Precompute per-block tile positions inside this gather:
            btiles = {}
            for ti in range(nt):
                b = int(tile_block[lo + ti])
                btiles.setdefault(b, []).append(ti)
            for b, tl in btiles.items():
                ka = int(kA[b])
                for w in range(P):
                    g0 = grp_start[(c * blocks + b) * P + w]
                    g1 = grp_end[(c * blocks + b) * P + w]
                    edges = src_r_s[g0:g1]
                    na = int(np.clip(ka, mA[b, w], max(cc[b, w] - mB[b, w], mA[b, w])))
                    na = min(na, ka, cc[b, w])
                    # ensure all non-A edges are B-eligible: put mustA first
                    ea = edges[edges < lo_max]
                    eb = edges[edges >= lo_max]
                    # A slots take from ea (must include all ea-only edges if
                    # B-ineligible). edges < b_off are A-only.
                    a_only = edges[edges < b_off]
                    both = edges[(edges >= b_off) & (edges < lo_max)]
                    b_only = eb
                    take_a = list(a_only) + list(both[: na - len(a_only)])
                    take_b = list(both[max(0, na - len(a_only)):]) + list(b_only)
                    if kind == 0:
                        for j, e in enumerate(take_a):
                            flat[tl[j] * P + w] = e
                            valid[tl[j] * P + w] = True
                    else:
                        for j, e in enumerate(take_b):
                            flat[tl[j] * P + w] = e - b_off
                            valid[tl[j] * P + w] = True
            # wrapped-16 layout
            wr = flat.reshape(nt * 8, 16).T.astype(np.int16)
            if kind == 0:
                idxA[:, col : col + nt * 8] = wr
            else:
                idxB[:, col : col + nt * 8] = wr
            # padbias (by global tile / lane)
            v = valid.reshape(nt, P)
            for ti in range(nt):
                gt = lo + ti
                padb[v[ti], gt] = 0.0

        percore.append(dict(
            idxA=np.tile(idxA, (8, 1)),
            idxB=np.tile(idxB, (8, 1)),
            padb=padb,
        ))

    # batch per (lane, block) and x permutation
    inv = np.empty(cores * npad, np.int64)  # table row -> orig node (or -1)
    inv[:] = -1
    inv[remap] = np.arange(N)
    for c in range(cores):
        rowsl = inv[c * npad : (c + 1) * npad]
        bb = np.full(npad, float(G), np.float32)
        ok = rowsl >= 0
        bb[ok] = batch[rowsl[ok]].astype(np.float32)
        percore[c]["batchb"] = bb.reshape(blocks, P).T.copy()  # [P, blocks]
        percore[c]["perm"] = rowsl                              # for x shard
    cnt_g = np.bincount(batch, minlength=G).astype(np.float32)
    sched["cnt_g"] = cnt_g
    return sched, percore


def build(cfg, sched, reps=None, parts="all", inner_reps=1):
    """inner_reps > 1: repeat the FULL inference (x load, L layers w/
    AllGather, pool, output write) inner_reps times as straight-line code
    (Python unroll — collectives inside a HW For_i crash NRT) — used to
    amortize the axon dispatch overhead when timing. inner_reps=1 is the
    production single-inference program.
    (reps/parts are the older partial-timing modes, kept for probes.)"""
    N, D, H, L, G = cfg["N"], cfg["D"], cfg["H"], cfg["L"], cfg["G"]
    cores = cfg["CORES"]
    npc, npad, blocks, rows, lo_max, b_off = _cfg_derived(cfg)
    T_TOT = sched["T_TOT"]
    RW = 2 * D  # table row elems (bf16): [h(D) | al_src(H) | pad]
    assert RW % 128 == 0

    nc = bacc.Bacc("TRN2", target_bir_lowering=False, debug=False, num_devices=cores,
                   num_swdge_queues=2)

    xs = nc.dram_tensor("xs", [npad, D], f32, kind="ExternalInput")
    wcat = nc.dram_tensor("wcat", [L, D, D + 2 * H], f32, kind="ExternalInput")
    bnsh = nc.dram_tensor("bnsh", [L, P, D], f32, kind="ExternalInput")
    idxA = nc.dram_tensor("idxA", [P, max(sched["colsA"], 8)], i16, kind="ExternalInput")
    idxB = nc.dram_tensor("idxB", [P, max(sched["colsB"], 8)], i16, kind="ExternalInput")
    padb = nc.dram_tensor("padb", [P, T_TOT], f32, kind="ExternalInput")
    batb = nc.dram_tensor("batchb", [P, blocks], f32, kind="ExternalInput")
    iog = nc.dram_tensor("iog", [P, G], f32, kind="ExternalInput")
    identb = nc.dram_tensor("identb", [P, P], bf16, kind="ExternalInput")
    identf = nc.dram_tensor("identf", [P, P], f32, kind="ExternalInput")
    outp = nc.dram_tensor("out", [G, D], f32, kind="ExternalOutput")

    slab = nc.dram_tensor("slab", [npad, RW], bf16)
    table = nc.dram_tensor("table", [rows, RW], bf16, addr_space="Shared")

    NAL = D + 2 * H  # 144 for D=128,H=4

    with tile.TileContext(nc) as tc:
        # ---- persistent SBUF ----
        x_sb = nc.alloc_sbuf_tensor("x_sb", [P, blocks * D], f32)
        ad_sb = nc.alloc_sbuf_tensor("ad_sb", [P, blocks * H], f32)  # al_dst per (lane, block)
        wc_sb = nc.alloc_sbuf_tensor("wc_sb", [P, L * NAL], f32)
        sh_sb = nc.alloc_sbuf_tensor("sh_sb", [P, L * D], f32)
        bat_sb = nc.alloc_sbuf_tensor("bat_sb", [P, blocks], f32)
        iog_sb = nc.alloc_sbuf_tensor("iog_sb", [P, G], f32)
        idb_sb = nc.alloc_sbuf_tensor("idb_sb", [P, P], bf16)
        idf_sb = nc.alloc_sbuf_tensor("idf_sb", [P, P], f32)
        pb_sb = nc.alloc_sbuf_tensor("pb_sb", [P, T_TOT], f32)

        if inner_reps == 1:
            nc.sync.dma_start(out=x_sb[:].rearrange("p (b f) -> p b f", f=D),
                              in_=xs[:, :].rearrange("(b p) f -> p b f", p=P))
        for l in range(L):
            nc.sync.dma_start(out=wc_sb[:, l * NAL : (l + 1) * NAL], in_=wcat[l])
            nc.sync.dma_start(out=sh_sb[:, l * D : (l + 1) * D], in_=bnsh[l])
        nc.sync.dma_start(out=bat_sb[:], in_=batb[:, :])
        nc.sync.dma_start(out=iog_sb[:], in_=iog[:, :])
        nc.sync.dma_start(out=idb_sb[:], in_=identb[:, :])
        nc.sync.dma_start(out=idf_sb[:], in_=identf[:, :])
        nc.sync.dma_start(out=pb_sb[:], in_=padb[:, :])

        from contextlib import nullcontext
        with (
            tc.tile_pool(name="sb", bufs=2) as sb,
            tc.tile_pool(name="sbg", bufs=2) as sbg,
            tc.tile_pool(name="ps", bufs=2, space="PSUM") as ps,
            tc.tile_pool(name="psacc", bufs=3, space="PSUM") as psacc,
            tc.tile_pool(name="pspool", bufs=1, space="PSUM") as pspool,
        ):
            def phase1_ag(l):
                for b in range(blocks):
                    xT_p = ps.tile([P, P], f32, tag="xT_p")
                    nc.tensor.transpose(out=xT_p[:], in_=x_sb[:, b * D : (b + 1) * D],
                                        identity=idf_sb[:])
                    xT_s = sb.tile([P, P], f32, tag="xT_s")
                    nc.vector.tensor_copy(out=xT_s[:], in_=xT_p[:])
                    hrow = ps.tile([P, NAL], f32, tag="hrow")
                    nc.tensor.matmul(hrow[:], lhsT=xT_s[:],
                                     rhs=wc_sb[:, l * NAL : (l + 1) * NAL],
                                     start=True, stop=True)
                    strow = sb.tile([P, RW], bf16, tag="strow")
                    nc.scalar.memzero(strow[:, D + H :])
                    nc.vector.tensor_copy(out=strow[:, 0 : D + H], in_=hrow[:, 0 : D + H])
                    nc.vector.tensor_copy(out=ad_sb[:, b * H : (b + 1) * H],
                                          in_=hrow[:, D + H : D + 2 * H])
                    nc.sync.dma_start(out=slab[b * P : (b + 1) * P, :], in_=strow[:])
                nc.gpsimd.collective_compute(
                    "AllGather", mybir.AluOpType.bypass,
                    replica_groups=[list(range(cores))],
                    ins=[slab[:, :].opt()], outs=[table[:, :].opt()],
                )
            if reps is not None:
                for l in range(L):
                    phase1_ag(l)
                loop_cm = tc.For_i(0, reps, 1)
                rep_range = [0]
            else:
                loop_cm = nullcontext()
                rep_range = range(inner_reps)
            with loop_cm:
              for rep in rep_range:
                if reps is None and inner_reps > 1:
                    nc.sync.dma_start(out=x_sb[:].rearrange("p (b f) -> p b f", f=D),
                                      in_=xs[:, :].rearrange("(b p) f -> p b f", p=P))
                for l in range(L):
                    # phase 1 (+ AG in normal mode; timing mode: phase1 only,
                    # AGs already done outside the loop)
                    for b in (range(blocks) if parts in ("all", "p1") else []):
                        xT_p = ps.tile([P, P], f32, tag="xT_p")
                        nc.tensor.transpose(out=xT_p[:], in_=x_sb[:, b * D : (b + 1) * D],
                                            identity=idf_sb[:])
                        xT_s = sb.tile([P, P], f32, tag="xT_s")
                        nc.vector.tensor_copy(out=xT_s[:], in_=xT_p[:])
                        hrow = ps.tile([P, NAL], f32, tag="hrow")
                        nc.tensor.matmul(hrow[:], lhsT=xT_s[:],
                                         rhs=wc_sb[:, l * NAL : (l + 1) * NAL],
                                         start=True, stop=True)
                        strow = sb.tile([P, RW], bf16, tag="strow")
                        nc.scalar.memzero(strow[:, D + H :])
                        nc.vector.tensor_copy(out=strow[:, 0 : D + H], in_=hrow[:, 0 : D + H])
                        nc.vector.tensor_copy(out=ad_sb[:, b * H : (b + 1) * H],
                                              in_=hrow[:, D + H : D + 2 * H])
                        nc.sync.dma_start(out=slab[b * P : (b + 1) * P, :], in_=strow[:])
                    if reps is None:
                        nc.gpsimd.collective_compute(
                            "AllGather", mybir.AluOpType.bypass,
                            replica_groups=[list(range(cores))],
                            ins=[slab[:, :].opt()], outs=[table[:, :].opt()],
                        )

                    if parts == "p1":
                        continue
                    if parts == "gather":
                        gdummy = sb.tile([P, 4], f32, tag="gdummy")
                        nc.vector.tensor_copy(out=gdummy[:], in_=x_sb[:, 0:4])
                    # ---------- edge phase ----------
                    # one G buffer per gather (per kind) so A/B run concurrently
                    # on separate SWDGE queues.
                    acc = {}          # block -> psum tile
                    gathers = sched["gathers"]
                    segs = sched["segs"]
                    for kind, lo, nt, col in gathers:
                        sc_hi = lo + nt
                        it = sb.tile([P, nt * 8], i16, tag=f"it{kind}")
                        src_idx = idxA if kind == 0 else idxB
                        nc.sync.dma_start(out=it[:], in_=src_idx[:, col : col + nt * 8])
                        in_ap = table[:, :] if kind == 0 else table[b_off:, :]
                        Gk = sbg.tile([P, nt * RW], bf16, tag=f"G{kind}")
                        nc.gpsimd.dma_gather(
                            out_ap=Gk[:].rearrange("p (j w) -> p j w", w=RW),
                            in_ap=in_ap,
                            idxs_ap=it[:],
                            num_idxs=nt * P,
                            num_idxs_reg=nt * P,
                            elem_size=RW,
                            single_packet=False,
                            queue_num=kind,
                        )
                        if parts == "gather":
                            nc.vector.tensor_tensor(out=gdummy[:], in0=gdummy[:],
                                                    in1=Gk[:, 0:4],
                                                    op=mybir.AluOpType.add)
                            continue
                        Z = sb.tile([P, nt * H], f32, tag=f"Z{kind}")
                        Gv = Gk[:].rearrange("p (t w) -> p t w", w=RW)
                        Zv = Z[:].rearrange("p (t h) -> p t h", h=H)
                        for (s_lo, s_hi, blk) in segs:
                            if s_hi <= lo or s_lo >= sc_hi:
                                continue
                            a, bnd = max(s_lo, lo) - lo, min(s_hi, sc_hi) - lo
                            nc.vector.tensor_tensor(
                                out=Zv[:, a:bnd, :],
                                in0=Gv[:, a:bnd, D : D + H],
                                in1=ad_sb[:, blk * H : (blk + 1) * H][:, None, :]
                                    .to_broadcast([P, bnd - a, H]),
                                op=mybir.AluOpType.add,
                            )
                        nc.vector.tensor_tensor(
                            out=Zv[:, :, :], in0=Zv[:, :, :],
                            in1=pb_sb[:, lo:sc_hi][:, :, None].to_broadcast([P, nt, H]),
                            op=mybir.AluOpType.add,
                        )
                        nc.vector.scalar_tensor_tensor(
                            out=Z[:], in0=Z[:], scalar=NEG_SLOPE, in1=Z[:],
                            op0=mybir.AluOpType.mult, op1=mybir.AluOpType.max,
                        )
                        nc.scalar.activation(out=Gv[:, :, D : D + H], in_=Zv[:, :, :],
                                             func=mybir.ActivationFunctionType.Exp)
                        nc.vector.tensor_tensor(
                            out=Gv[:, :, 0:D].rearrange("p t (h c) -> p t h c", h=H),
                            in0=Gv[:, :, 0:D].rearrange("p t (h c) -> p t h c", h=H),
                            in1=Gv[:, :, D : D + H][:, :, :, None]
                                .to_broadcast([P, nt, H, D // H]),
                            op=mybir.AluOpType.mult,
                        )
                        # matmuls + flushes for this gather's tiles
                        for t in range(lo, sc_hi):
                            blk = int(sched["tiles"][t][0])
                            if sched["first"][blk] == t:
                                acc_t = psacc.tile([P, D + H], f32, tag="acc")
                                acc[blk] = acc_t
                            nc.tensor.matmul(
                                acc[blk][:],
                                lhsT=idb_sb[:],
                                rhs=Gk[:, (t - lo) * RW : (t - lo) * RW + D + H],
                                start=(sched["first"][blk] == t),
                                stop=(sched["last"][blk] == t),
                            )
                            if sched["last"][blk] == t:
                                a = acc.pop(blk)
                                # flush: normalize + shift + ELU -> x_sb
                                s4 = sb.tile([P, H], f32, tag="s4")
                                nc.vector.tensor_scalar(
                                    out=s4[:], in0=a[:, D : D + H],
                                    scalar1=1e-16, scalar2=None,
                                    op0=mybir.AluOpType.add,
                                )
                                r4 = sb.tile([P, H], f32, tag="r4")
                                nc.vector.reciprocal(out=r4[:], in_=s4[:])
                                xb = x_sb[:, blk * D : (blk + 1) * D]
                                t0 = sb.tile([P, D], f32, tag="t0")
                                nc.vector.tensor_tensor(
                                    out=t0[:].rearrange("p (h c) -> p h c", h=H),
                                    in0=a[:, 0:D].rearrange("p (h c) -> p h c", h=H),
                                    in1=r4[:][:, :, None].to_broadcast([P, H, D // H]),
                                    op=mybir.AluOpType.mult,
                                )
                                # t1 = t0 + shift
                                nc.vector.tensor_tensor(
                                    out=t0[:], in0=t0[:],
                                    in1=sh_sb[:, l * D : (l + 1) * D],
                                    op=mybir.AluOpType.add,
                                )
                                # ELU: x = (max(t1,0)-1) + exp(min(t1,0))
                                mneg = sb.tile([P, D], f32, tag="mneg")
                                nc.vector.tensor_scalar(
                                    out=mneg[:], in0=t0[:], scalar1=0.0, scalar2=None,
                                    op0=mybir.AluOpType.min,
                                )
                                nc.scalar.activation(out=mneg[:], in_=mneg[:],
                                                     func=mybir.ActivationFunctionType.Exp)
                                nc.vector.tensor_scalar(
                                    out=xb, in0=t0[:], scalar1=0.0, scalar2=-1.0,
                                    op0=mybir.AluOpType.max, op1=mybir.AluOpType.add,
                                )
                                nc.vector.tensor_tensor(
                                    out=xb, in0=xb, in1=mneg[:], op=mybir.AluOpType.add,
                                )

                # ---------- mean-pool partials (inside loop for inner_reps) ----
                if reps is None and parts == "all":
                    pacc = pspool.tile([G, D], f32, tag="pacc")
                    for b in range(blocks):
                        sp = sb.tile([P, G], f32, tag="sp")
                        nc.vector.tensor_tensor(
                            out=sp[:],
                            in0=bat_sb[:, b : b + 1].to_broadcast([P, G]),
                            in1=iog_sb[:],
                            op=mybir.AluOpType.is_equal,
                        )
                        nc.tensor.matmul(pacc[:], lhsT=sp[:], rhs=x_sb[:, b * D : (b + 1) * D],
                                         start=(b == 0), stop=(b == blocks - 1))
                    po = sb.tile([G, D], f32, tag="po")
                    nc.vector.tensor_copy(out=po[:], in_=pacc[:])
                    nc.sync.dma_start(out=outp[:, :], in_=po[:])

    nc.compile()
    return nc


def _host_params(cfg, Ws, att_src, att_dst, bias, bn_gamma, bn_beta, bn_mean, bn_var):
    L, D, H = cfg["L"], cfg["D"], cfg["H"]
    C = D // H
    wcat = np.zeros((L, D, D + 2 * H), np.float32)
    bnsh = np.zeros((L, P, D), np.float32)
    for l in range(L):
        sc = bn_gamma[l] / np.sqrt(bn_var[l] + BN_EPS)
        sh = (bias[l] - bn_mean[l]) * sc + bn_beta[l]
        As = np.zeros((D, H), np.float32)
        Ad = np.zeros((D, H), np.float32)
        for h in range(H):
            As[h * C : (h + 1) * C, h] = att_src[l, h]
            Ad[h * C : (h + 1) * C, h] = att_dst[l, h]
        wcat[l, :, :D] = Ws[l] * sc[None, :]
        wcat[l, :, D : D + H] = Ws[l] @ As
        wcat[l, :, D + H : D + 2 * H] = Ws[l] @ Ad
        bnsh[l, :, :] = np.tile(sh[None, :], (P, 1))
    return wcat, bnsh


def _make_exec(nc, cores):
    """Build a reusable jitted executor for an nc (jit once, reuse forever)."""
    import jax
    from jax.sharding import Mesh, PartitionSpec
    from jax.experimental.shard_map import shard_map
    from concourse.bass2jax import (_bass_exec_p, install_neuronx_cc_hook,
                                    partition_id_tensor)

    install_neuronx_cc_hook()
    partition_name = nc.partition_id_tensor.name if nc.partition_id_tensor else None
    in_names, out_names, out_avals, zero_outs = [], [], [], []
    for alloc in nc.m.functions[0].allocations:
        if not isinstance(alloc, mybir.MemoryLocationSet):
            continue
        name = alloc.memorylocations[0].name
        if alloc.kind == "ExternalInput":
            if name != partition_name:
                in_names.append(name)
        elif alloc.kind == "ExternalOutput":
            out_names.append(name)
            shape = tuple(alloc.tensor_shape)
            dtype = mybir.dt.np(alloc.dtype)
            out_avals.append(jax.core.ShapedArray(shape, dtype))
            zero_outs.append(np.zeros(shape, dtype))
    n_params = len(in_names)
    n_outs = len(out_avals)
    in_names_full = in_names + out_names
    if partition_name is not None:
        in_names_full.append(partition_name)

    def _body(*args):
        operands = list(args)
        if partition_name is not None:
            operands.append(partition_id_tensor())
        outs = _bass_exec_p.bind(
            *operands,
            out_avals=tuple(out_avals),
            in_names=tuple(in_names_full),
            out_names=tuple(out_names),
            lowering_input_output_aliases=(),
            sim_require_finite=True,
            sim_require_nnan=True,
            nc=nc,
        )
        return tuple(outs)

    devices = jax.devices()[:cores]
    mesh = Mesh(np.asarray(devices), ("core",))
    in_specs = (PartitionSpec("core"),) * (n_params + n_outs)
    out_specs = (PartitionSpec("core"),) * len(out_names)
    donate = tuple(range(n_params, n_params + n_outs))
    fn = jax.jit(
        shard_map(_body, mesh=mesh, in_specs=in_specs, out_specs=out_specs,
                  check_rep=False),
        donate_argnums=donate,
        keep_unused=True,
    )
    return dict(fn=fn, in_names=in_names, out_names=out_names,
                zero_outs=zero_outs, mesh=mesh, cores=cores)


class Session:
    """Holds plan + compiled kernel + device-resident inputs for repeat runs."""

    def __init__(self, cfg, inputs, inner_reps=1, sched=None, percore=None):
        import jax
        from jax.sharding import NamedSharding, PartitionSpec
        self.cfg = cfg
        cores = cfg["CORES"]
        N, D, G = cfg["N"], cfg["D"], cfg["G"]
        npc, npad, blocks, rows, lo_max, b_off = _cfg_derived(cfg)
        if sched is None:
            sched, percore = plan(cfg, np.asarray(inputs["edge_index"]),
                                  np.asarray(inputs["batch"]))
        self.sched, self.percore = sched, percore
        self.nc = build(cfg, sched, inner_reps=inner_reps)
        self.exe = _make_exec(self.nc, cores)

        wcat, bnsh = _host_params(
            cfg, np.asarray(inputs["Ws"], np.float32),
            np.asarray(inputs["att_src"], np.float32),
            np.asarray(inputs["att_dst"], np.float32),
            np.asarray(inputs["bias"], np.float32),
            np.asarray(inputs["bn_gamma"], np.float32),
            np.asarray(inputs["bn_beta"], np.float32),
            np.asarray(inputs["bn_mean"], np.float32),
            np.asarray(inputs["bn_var"], np.float32),
        )
        iog = np.tile(np.arange(G, dtype=np.float32)[None, :], (P, 1))
        identf = np.eye(P, dtype=np.float32)
        import jax.numpy as jnp
        identb = np.asarray(jnp.asarray(identf, jnp.bfloat16))
        x = np.asarray(inputs["x"], np.float32)
        in_maps = []
        for c in range(cores):
            pc = percore[c]
            xs = np.zeros((npad, D), np.float32)
            ok = pc["perm"] >= 0
            xs[ok] = x[pc["perm"][ok]]
            in_maps.append(dict(
                xs=xs, wcat=wcat, bnsh=bnsh,
                idxA=pc["idxA"] if pc["idxA"].shape[1] else np.zeros((P, 8), np.int16),
                idxB=pc["idxB"] if pc["idxB"].shape[1] else np.zeros((P, 8), np.int16),
                padb=pc["padb"], batchb=pc["batchb"].astype(np.float32),
                iog=iog, identb=identb, identf=identf,
            ))
        sh = NamedSharding(self.exe["mesh"], PartitionSpec("core"))
        self.dev_in = [
            jax.device_put(
                np.concatenate([np.asarray(m[nm]) for m in in_maps], axis=0), sh)
            for nm in self.exe["in_names"]
        ]
        jax.block_until_ready(self.dev_in)

    def run(self):
        """One dispatch. Returns full [G, D] output (host)."""
        cores = self.exe["cores"]
        G, D = self.cfg["G"], self.cfg["D"]
        zo = [np.zeros((cores * z.shape[0], *z.shape[1:]), z.dtype)
              for z in self.exe["zero_outs"]]
        outs = self.exe["fn"](*self.dev_in, *zo)
        return self._reduce(outs)

    def _reduce(self, outs):
        cores = self.exe["cores"]
        G, D = self.cfg["G"], self.cfg["D"]
        parts = np.asarray(outs[0]).reshape(cores, G, D)
        out = parts.sum(axis=0) / np.maximum(self.sched["cnt_g"], 1.0)[:, None]
        return out.astype(np.float32)

    def bench(self, ndispatch):
        """Pipelined async timing: ndispatch launches, one sync. The donated
        output buffers are chained (call i's outputs seed call i+1) so no
        host->device traffic occurs inside the timed region. Returns
        (total_wall_seconds, host_output_of_last_dispatch)."""
        import time as _time
        import jax
        cores = self.exe["cores"]
        zo = [np.zeros((cores * z.shape[0], *z.shape[1:]), z.dtype)
              for z in self.exe["zero_outs"]]
        outs = self.exe["fn"](*self.dev_in, *zo)  # prime the chain
        jax.block_until_ready(outs)
        t0 = _time.time()
        for _ in range(ndispatch):
            outs = self.exe["fn"](*self.dev_in, *outs)
        jax.block_until_ready(outs)
        wall = _time.time() - t0
        return wall, self._reduce(outs)


def kernel(**inputs) -> np.ndarray:
    return Session(CFG_FULL, inputs).run()



# revision 12
# speedup vs baseline: 592.3488x; 1.2665x over previous
"""GAT encoder (3-layer, 4-head, BN+ELU, mean-pool) on 8 Trainium2 NeuronCores.

Self-contained: host-side planning (edge->slot assignment) + Bass/Tile kernel +
SPMD execution via a cached PJRT executor.

Design:
  - dst-shard nodes across 8 cores (5000/core, padded to 5120 = 40 blocks of 128).
  - Within a core, nodes are sorted by in-degree (desc) into (block, lane): the
    per-lane max over a block ~= the block's mean degree, so lane-aligned slot
    packing is dense.
  - Edge slot (tile, lane): lane = dst's lane; a tile is 128 slots; per block,
    tiles split into A-kind (table rows [0,32768)) and B-kind (rows [8192,40960)
    = offset view) so int16 dma_gather indices cover all 40960 rows; per-lane
    A/B assignment is balanced so pads are rare (pad slots gather row 0 and are
    killed by a -30000 bias before exp).
  - Table rows are 256B (h only, 128 bf16): al_src is NOT gathered; it is
    recomputed per edge from the gathered h via a per-head dot with
    asrc_eff = att_src / bn_scale (the BN scale is folded into W, so the
    gathered h is scaled; dividing asrc by the same scale cancels it in the
    logit). This halves both gather and AllGather traffic vs 512B rows.
  - Per layer: phase1 computes h|al_dst = x @ [W*bnscale | W@Adst] per block
    (PE transpose + bf16 matmul), rows -> local slab -> AllGather -> bf16 table
    [40960, 128]; edge phase gathers 256B rows by src across 4 SWDGE queues
    (each gather split in half), computes al_src on-device (mult+reduce),
    p = exp(leakyrelu(al_src + al_dst) + padbias), writes [h*p | p] (bf16) and
    accumulates into PSUM via identity-lhsT matmuls (segment sum); flush
    normalizes by s (softmax max-shift cancels; s+1e-16 guards padded rows),
    adds BN shift, applies ELU -> next layer's x (bf16 for layers 0..L-2, f32
    for the last layer which feeds the pool).
  - Mean-pool partials [64, 128] per core via one-hot matmuls; host sums across
    cores and divides by counts.

Timing note: the axon tunnel costs ~80-130ms per synchronous dispatch
regardless of kernel size. Session.bench amortizes it by pipelining many
async dispatches (donated output buffers chained device-side), and build()
supports inner_reps>1 to unroll several full inferences per dispatch
(straight-line: collectives inside a HW For_i crash NRT).
"""
import sys

sys.path.insert(0, "/opt/trn_rl_repo")

import numpy as np
from concourse import bass, mybir, tile, bacc

f32 = mybir.dt.float32
bf16 = mybir.dt.bfloat16
i16 = mybir.dt.int16

P = 128
NEG_SLOPE = 0.2
BN_EPS = 1e-5
PAD_BIAS = -30000.0

CFG_FULL = dict(N=40000, E=640000, D=128, H=4, L=3, G=64, CORES=8)


def _cfg_derived(cfg):
    cores = cfg["CORES"]
    npc = cfg["N"] // cores              # real nodes per core
    npad = -(-npc // P) * P              # padded nodes per core
    blocks = npad // P
    rows = npad * cores                  # global table rows
    lo_max = 32768                       # A-kind covers [0, lo_max)
    b_off = rows - 32768 if rows > 32768 else 0   # B-kind covers [b_off, rows)
    return npc, npad, blocks, rows, lo_max, b_off


def plan(cfg, edge_index, batch):
    """Host planning. Returns (sched, percore) where sched is core-uniform."""
    N, E, G = cfg["N"], cfg["E"], cfg["G"]
    cores = cfg["CORES"]
    npc, npad, blocks, rows, lo_max, b_off = _cfg_derived(cfg)

    src = np.asarray(edge_index[0], dtype=np.int64)
    dst = np.asarray(edge_index[1], dtype=np.int64)
    loops = np.arange(N, dtype=np.int64)
    src = np.concatenate([src, loops])
    dst = np.concatenate([dst, loops])
    batch = np.asarray(batch, dtype=np.int64)

    deg = np.bincount(dst, minlength=N)

    # node -> (core, block, lane); within a core sort by degree desc
    core_of = dst // npc  # for edges
    node_core = np.arange(N) // npc
    pos = np.empty(N, dtype=np.int64)       # position within core (block*128+lane)
    for c in range(cores):
        nodes = np.arange(c * npc, (c + 1) * npc)
        order = nodes[np.argsort(-deg[nodes], kind="stable")]
        pos[order] = np.arange(npc)
    remap = node_core * npad + pos          # node -> global table row

    src_r = remap[src]                      # gather row of each edge
    dst_c = core_of                         # owning core of each edge
    dst_b = pos[dst] // P                   # block within core
    dst_w = pos[dst] % P                    # lane

    # group edges by (core, block, lane)
    key = (dst_c * blocks + dst_b) * P + dst_w
    order = np.argsort(key, kind="stable")
    src_r_s = src_r[order]
    key_s = key[order]
    grp_start = np.searchsorted(key_s, np.arange(cores * blocks * P))
    grp_end = np.searchsorted(key_s, np.arange(cores * blocks * P) + 1)

    # per (core, block, lane): how many edges must be A (src_row < lo_max can be
    # A; src_row >= b_off can be B). mustA: src_row < b_off; mustB: >= lo_max.
    is_mustA = src_r_s < b_off
    is_mustB = src_r_s >= lo_max
    mustA = np.zeros(cores * blocks * P, np.int64)
    mustB = np.zeros(cores * blocks * P, np.int64)
    np.add.at(mustA, key_s, is_mustA)
    np.add.at(mustB, key_s, is_mustB)
    cnt = grp_end - grp_start

    mustA = mustA.reshape(cores, blocks, P)
    mustB = mustB.reshape(cores, blocks, P)
    cntr = cnt.reshape(cores, blocks, P)

    # choose per-block (shared across cores) k_A, k_B minimizing pads
    kA = np.zeros(blocks, np.int64)
    kB = np.zeros(blocks, np.int64)
    for b in range(blocks):
        mA, mB, cc = mustA[:, b], mustB[:, b], cntr[:, b]
        best = None
        lo = int(mA.max())
        hi = int(np.maximum(cc - mB, mA).max())
        for ka in range(lo, hi + 1):
            nA = np.clip(ka, mA, np.maximum(cc - mB, mA))
            nA = np.minimum(nA, ka)  # lane can't use more A slots than exist
            nA = np.maximum(nA, np.minimum(mA, ka))
            # feasibility: every lane must fit its edges: nB = cc - nA <= kb
            nB = cc - nA
            kb = int(nB.max())
            pads = (ka - nA).sum() + (kb - nB).sum()
            if best is None or pads < best[0]:
                best = (pads, ka, kb)
        _, ka, kb = best
        kA[b], kB[b] = ka, kb

    # global tile layout: superchunks of SC blocks; within: A tiles of the
    # blocks (in block order), then B tiles of the blocks.
    SC = 2
    tiles = []          # list of (block, kind)
    gathers = []        # list of (kind, tile_lo, tile_hi)  [tile indices into `tiles`]
    t = 0
    for s0 in range(0, blocks, SC):
        bl = list(range(s0, min(s0 + SC, blocks)))
        a0 = t
        for b in bl:
            tiles += [(b, 0)] * int(kA[b])
        t = len(tiles)
        if t > a0:
            gathers.append((0, a0, t))
        b0 = t
        for b in bl:
            tiles += [(b, 1)] * int(kB[b])
        t = len(tiles)
        if t > b0:
            gathers.append((1, b0, t))
    T_TOT = len(tiles)
    tile_block = np.array([b for b, _ in tiles], np.int64)
    # first/last tile per block
    first = {}
    last = {}
    for i, (b, _) in enumerate(tiles):
        if b not in first:
            first[b] = i
        last[b] = i

    # z-chain segments: runs of equal block in tile order
    segs = []  # (tile_lo, tile_hi, block)
    i = 0
    while i < T_TOT:
        j = i
        while j < T_TOT and tile_block[j] == tile_block[i]:
            j += 1
        segs.append((i, j, int(tile_block[i])))
        i = j

    # per-gather idx column offsets (in int16 columns, each tile -> 8 columns)
    gmeta = []
    colA = colB = 0
    for kind, lo, hi in gathers:
        nt = hi - lo
        if kind == 0:
            gmeta.append((kind, lo, nt, colA))
            colA += nt * 8
        else:
            gmeta.append((kind, lo, nt, colB))
            colB += nt * 8

    sched = dict(
        T_TOT=T_TOT, tiles=tiles, gathers=gmeta, segs=segs,
        first=first, last=last, kA=kA, kB=kB,
        colsA=colA, colsB=colB, blocks=blocks, npad=npad, rows=rows,
        b_off=b_off,
    )

    # ---------- per-core data ----------
    percore = []
    for c in range(cores):
        idxA = np.zeros((16, colA), np.int16)
        idxB = np.zeros((16, colB), np.int16)
        padb = np.full((P, T_TOT), PAD_BIAS, np.float32)
        # slot fill: per block, per lane: A-edges then B-edges of that lane
        # (choose nA per lane as planned)
        mA, mB, cc = mustA[c], mustB[c], cntr[c]
        for kind, lo, nt, col in gmeta:
            flat = np.zeros(nt * P, np.int64)   # default pad -> row 0
            valid = np.zeros(nt * P, bool)
            btiles = {}
            for ti in range(nt):
                b = int(tile_block[lo + ti])
                btiles.setdefault(b, []).append(ti)
            for b, tl in btiles.items():
                ka = int(kA[b])
                for w in range(P):
                    g0 = grp_start[(c * blocks + b) * P + w]
                    g1 = grp_end[(c * blocks + b) * P + w]
                    edges = src_r_s[g0:g1]
                    na = int(np.clip(ka, mA[b, w], max(cc[b, w] - mB[b, w], mA[b, w])))
                    na = min(na, ka, cc[b, w])
                    a_only = edges[edges < b_off]
                    both = edges[(edges >= b_off) & (edges < lo_max)]
                    b_only = edges[edges >= lo_max]
                    take_a = list(a_only) + list(both[: na - len(a_only)])
                    take_b = list(both[max(0, na - len(a_only)):]) + list(b_only)
                    if kind == 0:
                        for j, e in enumerate(take_a):
                            flat[tl[j] * P + w] = e
                            valid[tl[j] * P + w] = True
                    else:
                        for j, e in enumerate(take_b):
                            flat[tl[j] * P + w] = e - b_off
                            valid[tl[j] * P + w] = True
            # wrapped-16 layout
            wr = flat.reshape(nt * 8, 16).T.astype(np.int16)
            if kind == 0:
                idxA[:, col : col + nt * 8] = wr
            else:
                idxB[:, col : col + nt * 8] = wr
            # padbias (by global tile / lane)
            v = valid.reshape(nt, P)
            for ti in range(nt):
                gt = lo + ti
                padb[v[ti], gt] = 0.0

        percore.append(dict(
            idxA=np.tile(idxA, (8, 1)),
            idxB=np.tile(idxB, (8, 1)),
            padb=padb,
        ))

    # batch per (lane, block) and x permutation
    inv = np.empty(cores * npad, np.int64)  # table row -> orig node (or -1)
    inv[:] = -1
    inv[remap] = np.arange(N)
    for c in range(cores):
        rowsl = inv[c * npad : (c + 1) * npad]
        bb = np.full(npad, float(G), np.float32)
        ok = rowsl >= 0
        bb[ok] = batch[rowsl[ok]].astype(np.float32)
        percore[c]["batchb"] = bb.reshape(blocks, P).T.copy()  # [P, blocks]
        percore[c]["perm"] = rowsl                              # for x shard
    cnt_g = np.bincount(batch, minlength=G).astype(np.float32)
    sched["cnt_g"] = cnt_g
    return sched, percore


def build(cfg, sched, reps=None, parts="all", inner_reps=1):
    """inner_reps > 1: repeat the FULL inference (x load, L layers w/
    AllGather, pool, output write) inner_reps times as straight-line code —
    used to amortize the axon dispatch overhead when timing. inner_reps=1 is
    the production single-inference program.
    parts: "all" | "p1x" (phase1 only) | "p1" (phase1+AG) | "gather"
    (AG+gathers, no phase1/edge-compute) — HW-time breakdown probes."""
    N, D, H, L, G = cfg["N"], cfg["D"], cfg["H"], cfg["L"], cfg["G"]
    cores = cfg["CORES"]
    npc, npad, blocks, rows, lo_max, b_off = _cfg_derived(cfg)
    T_TOT = sched["T_TOT"]
    RW = D          # table row elems (bf16): h only -> 256B rows
    NAL = D + H     # phase1 matmul cols: [h | al_dst]

    nc = bacc.Bacc("TRN2", target_bir_lowering=False, debug=False, num_devices=cores,
                   num_swdge_queues=4)

    xsb = nc.dram_tensor("xsb", [npad, D], bf16, kind="ExternalInput")
    wcat = nc.dram_tensor("wcat", [L, D, NAL], bf16, kind="ExternalInput")
    asrc = nc.dram_tensor("asrc", [P, L * D], bf16, kind="ExternalInput")
    bnsh = nc.dram_tensor("bnsh", [L, P, D], f32, kind="ExternalInput")
    idxA = nc.dram_tensor("idxA", [P, max(sched["colsA"], 8)], i16, kind="ExternalInput")
    idxB = nc.dram_tensor("idxB", [P, max(sched["colsB"], 8)], i16, kind="ExternalInput")
    padb = nc.dram_tensor("padb", [P, T_TOT], f32, kind="ExternalInput")
    batb = nc.dram_tensor("batchb", [P, blocks], f32, kind="ExternalInput")
    iog = nc.dram_tensor("iog", [P, G], f32, kind="ExternalInput")
    identb = nc.dram_tensor("identb", [P, P], bf16, kind="ExternalInput")
    outp = nc.dram_tensor("out", [G, D], f32, kind="ExternalOutput")

    slab = nc.dram_tensor("slab", [npad, RW], bf16)
    table = nc.dram_tensor("table", [rows, RW], bf16, addr_space="Shared")

    with tile.TileContext(nc) as tc:
        # ---- persistent SBUF ----
        x_sb = nc.alloc_sbuf_tensor("x_sb", [P, blocks * D], f32)    # last layer -> pool
        x_bf = nc.alloc_sbuf_tensor("x_bf", [P, blocks * D], bf16)   # phase1 input
        ad_sb = nc.alloc_sbuf_tensor("ad_sb", [P, blocks * H], f32)  # al_dst per (lane, block)
        wc_sb = nc.alloc_sbuf_tensor("wc_sb", [P, L * NAL], bf16)
        as_sb = nc.alloc_sbuf_tensor("as_sb", [P, L * D], bf16)
        sh_sb = nc.alloc_sbuf_tensor("sh_sb", [P, L * D], f32)
        bat_sb = nc.alloc_sbuf_tensor("bat_sb", [P, blocks], f32)
        iog_sb = nc.alloc_sbuf_tensor("iog_sb", [P, G], f32)
        idb_sb = nc.alloc_sbuf_tensor("idb_sb", [P, P], bf16)
        pb_sb = nc.alloc_sbuf_tensor("pb_sb", [P, T_TOT], f32)

        for l in range(L):
            nc.sync.dma_start(out=wc_sb[:, l * NAL : (l + 1) * NAL], in_=wcat[l])
            nc.sync.dma_start(out=sh_sb[:, l * D : (l + 1) * D], in_=bnsh[l])
        nc.sync.dma_start(out=as_sb[:], in_=asrc[:, :])
        nc.sync.dma_start(out=bat_sb[:], in_=batb[:, :])
        nc.sync.dma_start(out=iog_sb[:], in_=iog[:, :])
        nc.sync.dma_start(out=idb_sb[:], in_=identb[:, :])
        nc.sync.dma_start(out=pb_sb[:], in_=padb[:, :])

        from contextlib import nullcontext
        with (
            tc.tile_pool(name="sb", bufs=2) as sb,
            tc.tile_pool(name="sbg", bufs=2) as sbg,
            tc.tile_pool(name="ps", bufs=2, space="PSUM") as ps,
            tc.tile_pool(name="psacc", bufs=3, space="PSUM") as psacc,
            tc.tile_pool(name="pspool", bufs=1, space="PSUM") as pspool,
        ):
            def phase1(l):
                for b in range(blocks):
                    xT_p = ps.tile([P, P], bf16, tag="xT_p")
                    nc.tensor.transpose(out=xT_p[:], in_=x_bf[:, b * D : (b + 1) * D],
                                        identity=idb_sb[:])
                    xT_s = sb.tile([P, P], bf16, tag="xT_s")
                    nc.vector.tensor_copy(out=xT_s[:], in_=xT_p[:])
                    hrow = ps.tile([P, NAL], f32, tag="hrow")
                    nc.tensor.matmul(hrow[:], lhsT=xT_s[:],
                                     rhs=wc_sb[:, l * NAL : (l + 1) * NAL],
                                     start=True, stop=True)
                    strow = sb.tile([P, D], bf16, tag="strow")
                    nc.vector.tensor_copy(out=strow[:], in_=hrow[:, 0:D])
                    nc.vector.tensor_copy(out=ad_sb[:, b * H : (b + 1) * H],
                                          in_=hrow[:, D : D + H])
                    nc.sync.dma_start(out=slab[b * P : (b + 1) * P, :], in_=strow[:])

            def allgather():
                nc.gpsimd.collective_compute(
                    "AllGather", mybir.AluOpType.bypass,
                    replica_groups=[list(range(cores))],
                    ins=[slab[:, :].opt()], outs=[table[:, :].opt()],
                )

            if reps is not None:
                for l in range(L):
                    phase1(l)
                    allgather()
                loop_cm = tc.For_i(0, reps, 1)
                rep_range = [0]
            else:
                loop_cm = nullcontext()
                rep_range = range(inner_reps)
            with loop_cm:
              for rep in rep_range:
                # per-inference x load (bf16)
                if reps is None:
                    nc.sync.dma_start(out=x_bf[:].rearrange("p (b f) -> p b f", f=D),
                                      in_=xsb[:, :].rearrange("(b p) f -> p b f", p=P))
                qn = [0]  # round-robin SWDGE queue counter
                for l in range(L):
                    if parts in ("all", "p1", "p1x"):
                        phase1(l)
                    if reps is None and parts != "p1x":
                        allgather()
                    if parts in ("p1", "p1x"):
                        continue
                    if parts == "gather":
                        gdummy = sb.tile([P, 4], f32, tag="gdummy")
                        nc.vector.tensor_copy(out=gdummy[:], in_=x_sb[:, 0:4])
                    # ---------- edge phase ----------
                    acc = {}          # block -> psum tile
                    gathers = sched["gathers"]
                    segs = sched["segs"]
                    for kind, lo, nt, col in gathers:
                        sc_hi = lo + nt
                        it = sb.tile([P, nt * 8], i16, tag=f"it{kind}")
                        src_idx = idxA if kind == 0 else idxB
                        nc.sync.dma_start(out=it[:], in_=src_idx[:, col : col + nt * 8])
                        in_ap = table[:, :] if kind == 0 else table[b_off:, :]
                        Gk = sbg.tile([P, nt * RW], bf16, tag=f"G{kind}")
                        # split into halves across SWDGE queues (round-robin)
                        h0 = nt // 2
                        pieces = [(0, h0), (h0, nt)] if h0 > 0 else [(0, nt)]
                        for (plo, phi) in pieces:
                            if phi <= plo:
                                continue
                            nc.gpsimd.dma_gather(
                                out_ap=Gk[:, plo * RW : phi * RW]
                                    .rearrange("p (j w) -> p j w", w=RW),
                                in_ap=in_ap,
                                idxs_ap=it[:, plo * 8 : phi * 8],
                                num_idxs=(phi - plo) * P,
                                num_idxs_reg=(phi - plo) * P,
                                elem_size=RW,
                                single_packet=False,
                                queue_num=qn[0] % 4,
                            )
                            qn[0] += 1
                        if parts == "gather":
                            nc.vector.tensor_tensor(out=gdummy[:], in0=gdummy[:],
                                                    in1=Gk[:, 0:4],
                                                    op=mybir.AluOpType.add)
                            continue
                        Gv = Gk[:].rearrange("p (t w) -> p t w", w=RW)
                        G4 = Gk[:].rearrange("p (t h c) -> p t h c", h=H, c=D // H)
                        # al_src from gathered h: tmp = h * asrc_eff; Z = sum_c
                        tmp = sb.tile([P, nt * D], bf16, tag=f"tmp{kind}")
                        t4 = tmp[:].rearrange("p (t h c) -> p t h c", h=H, c=D // H)
                        a4 = (as_sb[:, l * D : (l + 1) * D]
                              .rearrange("p (h c) -> p h c", h=H)[:, None, :, :]
                              .to_broadcast([P, nt, H, D // H]))
                        nc.vector.tensor_tensor(out=t4, in0=G4, in1=a4,
                                                op=mybir.AluOpType.mult)
                        Z = sb.tile([P, nt * H], f32, tag=f"Z{kind}")
                        Zv = Z[:].rearrange("p (t h) -> p t h", h=H)
                        nc.vector.tensor_reduce(out=Zv, in_=t4,
                                                axis=mybir.AxisListType.X,
                                                op=mybir.AluOpType.add)
                        # + al_dst (per block segment) + pad bias
                        for (s_lo, s_hi, blk) in segs:
                            if s_hi <= lo or s_lo >= sc_hi:
                                continue
                            a, bnd = max(s_lo, lo) - lo, min(s_hi, sc_hi) - lo
                            nc.vector.tensor_tensor(
                                out=Zv[:, a:bnd, :],
                                in0=Zv[:, a:bnd, :],
                                in1=ad_sb[:, blk * H : (blk + 1) * H][:, None, :]
                                    .to_broadcast([P, bnd - a, H]),
                                op=mybir.AluOpType.add,
                            )
                        nc.vector.tensor_tensor(
                            out=Zv[:, :, :], in0=Zv[:, :, :],
                            in1=pb_sb[:, lo:sc_hi][:, :, None].to_broadcast([P, nt, H]),
                            op=mybir.AluOpType.add,
                        )
                        nc.vector.scalar_tensor_tensor(
                            out=Z[:], in0=Z[:], scalar=NEG_SLOPE, in1=Z[:],
                            op0=mybir.AluOpType.mult, op1=mybir.AluOpType.max,
                        )
                        # W_t = [h*p | p] (bf16)
                        Wt = sb.tile([P, nt * NAL], bf16, tag=f"W{kind}")
                        Wv = Wt[:].rearrange("p (t w) -> p t w", w=NAL)
                        nc.scalar.activation(out=Wv[:, :, D : D + H], in_=Zv[:, :, :],
                                             func=mybir.ActivationFunctionType.Exp)
                        nc.vector.tensor_tensor(
                            out=Wv[:, :, 0:D].rearrange("p t (h c) -> p t h c", h=H),
                            in0=G4,
                            in1=Wv[:, :, D : D + H][:, :, :, None]
                                .to_broadcast([P, nt, H, D // H]),
                            op=mybir.AluOpType.mult,
                        )
                        # matmuls + flushes for this gather's tiles
                        for t in range(lo, sc_hi):
                            blk = int(sched["tiles"][t][0])
                            if sched["first"][blk] == t:
                                acc_t = psacc.tile([P, NAL], f32, tag="acc")
                                acc[blk] = acc_t
                            nc.tensor.matmul(
                                acc[blk][:],
                                lhsT=idb_sb[:],
                                rhs=Wt[:, (t - lo) * NAL : (t - lo) * NAL + NAL],
                                start=(sched["first"][blk] == t),
                                stop=(sched["last"][blk] == t),
                            )
                            if sched["last"][blk] == t:
                                a = acc.pop(blk)
                                # flush: normalize + shift + ELU -> next x
                                s4 = sb.tile([P, H], f32, tag="s4")
                                nc.vector.tensor_scalar(
                                    out=s4[:], in0=a[:, D : D + H],
                                    scalar1=1e-16, scalar2=None,
                                    op0=mybir.AluOpType.add,
                                )
                                r4 = sb.tile([P, H], f32, tag="r4")
                                nc.vector.reciprocal(out=r4[:], in_=s4[:])
                                t0 = sb.tile([P, D], f32, tag="t0")
                                nc.vector.tensor_tensor(
                                    out=t0[:].rearrange("p (h c) -> p h c", h=H),
                                    in0=a[:, 0:D].rearrange("p (h c) -> p h c", h=H),
                                    in1=r4[:][:, :, None].to_broadcast([P, H, D // H]),
                                    op=mybir.AluOpType.mult,
                                )
                                # t0 += BN shift
                                nc.vector.tensor_tensor(
                                    out=t0[:], in0=t0[:],
                                    in1=sh_sb[:, l * D : (l + 1) * D],
                                    op=mybir.AluOpType.add,
                                )
                                # ELU: x = (max(t0,0)-1) + exp(min(t0,0))
                                mneg = sb.tile([P, D], f32, tag="mneg")
                                nc.vector.tensor_scalar(
                                    out=mneg[:], in0=t0[:], scalar1=0.0, scalar2=None,
                                    op0=mybir.AluOpType.min,
                                )
                                nc.scalar.activation(out=mneg[:], in_=mneg[:],
                                                     func=mybir.ActivationFunctionType.Exp)
                                xb = (x_sb[:, blk * D : (blk + 1) * D] if l == L - 1
                                      else x_bf[:, blk * D : (blk + 1) * D])
                                nc.vector.tensor_scalar(
                                    out=xb, in0=t0[:], scalar1=0.0, scalar2=-1.0,
                                    op0=mybir.AluOpType.max, op1=mybir.AluOpType.add,
                                )
                                nc.vector.tensor_tensor(
                                    out=xb, in0=xb, in1=mneg[:], op=mybir.AluOpType.add,
                                )

                # ---------- mean-pool partials ----------
                if reps is None and parts == "all":
                    pacc = pspool.tile([G, D], f32, tag="pacc")
                    for b in range(blocks):
                        sp = sb.tile([P, G], f32, tag="sp")
                        nc.vector.tensor_tensor(
                            out=sp[:],
                            in0=bat_sb[:, b : b + 1].to_broadcast([P, G]),
                            in1=iog_sb[:],
                            op=mybir.AluOpType.is_equal,
                        )
                        nc.tensor.matmul(pacc[:], lhsT=sp[:], rhs=x_sb[:, b * D : (b + 1) * D],
                                         start=(b == 0), stop=(b == blocks - 1))
                    po = sb.tile([G, D], f32, tag="po")
                    nc.vector.tensor_copy(out=po[:], in_=pacc[:])
                    nc.sync.dma_start(out=outp[:, :], in_=po[:])

    nc.compile()
    return nc


def _host_params(cfg, Ws, att_src, att_dst, bias, bn_gamma, bn_beta, bn_mean, bn_var):
    L, D, H = cfg["L"], cfg["D"], cfg["H"]
    C = D // H
    NAL = D + H
    wcat = np.zeros((L, D, NAL), np.float32)
    bnsh = np.zeros((L, P, D), np.float32)
    asrc_eff = np.zeros((L, D), np.float32)
    for l in range(L):
        sc = bn_gamma[l] / np.sqrt(bn_var[l] + BN_EPS)
        sh = (bias[l] - bn_mean[l]) * sc + bn_beta[l]
        Ad = np.zeros((D, H), np.float32)
        for h in range(H):
            Ad[h * C : (h + 1) * C, h] = att_dst[l, h]
        wcat[l, :, :D] = Ws[l] * sc[None, :]
        wcat[l, :, D : D + H] = Ws[l] @ Ad
        bnsh[l, :, :] = np.tile(sh[None, :], (P, 1))
        # gathered h is scaled by sc; divide asrc by sc so the logit is exact
        asrc_eff[l] = att_src[l].reshape(D) / sc
    return wcat, bnsh, asrc_eff


def _make_exec(nc, cores):
    """Build a reusable jitted executor for an nc (jit once, reuse forever)."""
    import jax
    from jax.sharding import Mesh, PartitionSpec
    from jax.experimental.shard_map import shard_map
    from concourse.bass2jax import (_bass_exec_p, install_neuronx_cc_hook,
                                    partition_id_tensor)

    install_neuronx_cc_hook()
    partition_name = nc.partition_id_tensor.name if nc.partition_id_tensor else None
    in_names, out_names, out_avals, zero_outs = [], [], [], []
    for alloc in nc.m.functions[0].allocations:
        if not isinstance(alloc, mybir.MemoryLocationSet):
            continue
        name = alloc.memorylocations[0].name
        if alloc.kind == "ExternalInput":
            if name != partition_name:
                in_names.append(name)
        elif alloc.kind == "ExternalOutput":
            out_names.append(name)
            shape = tuple(alloc.tensor_shape)
            dtype = mybir.dt.np(alloc.dtype)
            out_avals.append(jax.core.ShapedArray(shape, dtype))
            zero_outs.append(np.zeros(shape, dtype))
    n_params = len(in_names)
    n_outs = len(out_avals)
    in_names_full = in_names + out_names
    if partition_name is not None:
        in_names_full.append(partition_name)

    def _body(*args):
        operands = list(args)
        if partition_name is not None:
            operands.append(partition_id_tensor())
        outs = _bass_exec_p.bind(
            *operands,
            out_avals=tuple(out_avals),
            in_names=tuple(in_names_full),
            out_names=tuple(out_names),
            lowering_input_output_aliases=(),
            sim_require_finite=True,
            sim_require_nnan=True,
            nc=nc,
        )
        return tuple(outs)

    devices = jax.devices()[:cores]
    mesh = Mesh(np.asarray(devices), ("core",))
    in_specs = (PartitionSpec("core"),) * (n_params + n_outs)
    out_specs = (PartitionSpec("core"),) * len(out_names)
    donate = tuple(range(n_params, n_params + n_outs))
    fn = jax.jit(
        shard_map(_body, mesh=mesh, in_specs=in_specs, out_specs=out_specs,
                  check_rep=False),
        donate_argnums=donate,
        keep_unused=True,
    )
    return dict(fn=fn, in_names=in_names, out_names=out_names,
                zero_outs=zero_outs, mesh=mesh, cores=cores)


class Session:
    """Holds plan + compiled kernel + device-resident inputs for repeat runs."""

    def __init__(self, cfg, inputs, inner_reps=1, sched=None, percore=None,
                 parts="all"):
        import jax
        import jax.numpy as jnp
        from jax.sharding import NamedSharding, PartitionSpec
        self.cfg = cfg
        cores = cfg["CORES"]
        N, D, G = cfg["N"], cfg["D"], cfg["G"]
        npc, npad, blocks, rows, lo_max, b_off = _cfg_derived(cfg)
        if sched is None:
            sched, percore = plan(cfg, np.asarray(inputs["edge_index"]),
                                  np.asarray(inputs["batch"]))
        self.sched, self.percore = sched, percore
        self.nc = build(cfg, sched, inner_reps=inner_reps, parts=parts)
        self.exe = _make_exec(self.nc, cores)

        wcat, bnsh, asrc_eff = _host_params(
            cfg, np.asarray(inputs["Ws"], np.float32),
            np.asarray(inputs["att_src"], np.float32),
            np.asarray(inputs["att_dst"], np.float32),
            np.asarray(inputs["bias"], np.float32),
            np.asarray(inputs["bn_gamma"], np.float32),
            np.asarray(inputs["bn_beta"], np.float32),
            np.asarray(inputs["bn_mean"], np.float32),
            np.asarray(inputs["bn_var"], np.float32),
        )

        def to_bf16(a):
            return np.asarray(jnp.asarray(np.asarray(a, np.float32), jnp.bfloat16))

        iog = np.tile(np.arange(G, dtype=np.float32)[None, :], (P, 1))
        identb = to_bf16(np.eye(P, dtype=np.float32))
        wcat_b = to_bf16(wcat)
        # asrc replicated over partitions: [P, L*D]
        asrc_b = to_bf16(np.tile(asrc_eff.reshape(1, -1), (P, 1)))
        x = np.asarray(inputs["x"], np.float32)
        in_maps = []
        for c in range(cores):
            pc = percore[c]
            xs = np.zeros((npad, D), np.float32)
            ok = pc["perm"] >= 0
            xs[ok] = x[pc["perm"][ok]]
            in_maps.append(dict(
                xsb=to_bf16(xs), wcat=wcat_b, asrc=asrc_b, bnsh=bnsh,
                idxA=pc["idxA"] if pc["idxA"].shape[1] else np.zeros((P, 8), np.int16),
                idxB=pc["idxB"] if pc["idxB"].shape[1] else np.zeros((P, 8), np.int16),
                padb=pc["padb"], batchb=pc["batchb"].astype(np.float32),
                iog=iog, identb=identb,
            ))
        sh = NamedSharding(self.exe["mesh"], PartitionSpec("core"))
        self.dev_in = [
            jax.device_put(
                np.concatenate([np.asarray(m[nm]) for m in in_maps], axis=0), sh)
            for nm in self.exe["in_names"]
        ]
        jax.block_until_ready(self.dev_in)

    def run(self):
        """One dispatch. Returns full [G, D] output (host)."""
        cores = self.exe["cores"]
        zo = [np.zeros((cores * z.shape[0], *z.shape[1:]), z.dtype)
              for z in self.exe["zero_outs"]]
        outs = self.exe["fn"](*self.dev_in, *zo)
        return self._reduce(outs)

    def _reduce(self, outs):
        cores = self.exe["cores"]
        G, D = self.cfg["G"], self.cfg["D"]
        parts = np.asarray(outs[0]).reshape(cores, G, D)
        out = parts.sum(axis=0) / np.maximum(self.sched["cnt_g"], 1.0)[:, None]
        return out.astype(np.float32)

    def bench(self, ndispatch):
        """Pipelined async timing: ndispatch launches, one sync. The donated
        output buffers are chained (call i's outputs seed call i+1) so no
        host->device traffic occurs inside the timed region. Returns
        (total_wall_seconds, host_output_of_last_dispatch)."""
        import time as _time
        import jax
        cores = self.exe["cores"]
        zo = [np.zeros((cores * z.shape[0], *z.shape[1:]), z.dtype)
              for z in self.exe["zero_outs"]]
        outs = self.exe["fn"](*self.dev_in, *zo)  # prime the chain
        jax.block_until_ready(outs)
        t0 = _time.time()
        for _ in range(ndispatch):
            outs = self.exe["fn"](*self.dev_in, *outs)
        jax.block_until_ready(outs)
        wall = _time.time() - t0
        return wall, self._reduce(outs)


def kernel(**inputs) -> np.ndarray:
    return Session(CFG_FULL, inputs).run()
